# revision 41
# baseline (speedup 1.0000x reference)
"""AKOrN layer on 8 TRN2 NeuronCores, data-parallel over batch.

reference: v = l2norm_d(x @ W_in); K = tanh(coupling);
           8x: v = l2norm_d(v + K @ v + omega); return v [B, OUT, D]

Implementation notes:
- Data-parallel: batch 8192 -> 1024 rows per core; W_in/coupling/omega
  replicated. No collectives.
- K' = tanh(coupling) + I folds the "+ v" into the step matmul, so each step
  is pure matmul work plus a PSUM-side normalize.
- v lives on-chip as 4 per-d planes [OUT(part), batch(free)] in fp16 (8x
  finer mantissa than bf16 at identical PE speed; bf16 landed at rel err
  2.4e-2, fp16 at 3.3e-3). Batch is processed in 2 sequential 512-column
  chunks (SBUF fit for the double-buffered v generations).
- Step: 8 j-tiles x 4 d matmuls accumulate K'^T @ v_d into 4 PSUM banks
  (2 normalize units in flight = all 8 banks), then:
    q_d = Square(psum_d + omega_d)        (ACT, per-partition bias, fp16 out)
    s   = q0+q1+q2+q3                     (DVE fp16, 2x mode)
    inv = Exp(-0.5 * Ln(s))               (ACT, == rsqrt(s), one act table)
    v'_d = (psum_d + omega_d) * inv       (DVE scalar_tensor_tensor -> fp16)
- Last step runs transposed (stationary = v-slice, moving = K'^T rows) to
  produce [batch, OUT] so the d-interleave + output DMA is contiguous;
  omega enters there via a K=1 ones-row matmul (skipped when omega == 0).
  u is staged out of PSUM by ACT copies so banks free early.
- chunk1's first matmul is emitted between chunk0's steps and last step:
  its matmuls fill the last-step PSUM-drain stalls.
- x^T and W_in are uploaded pre-transposed/de-interleaved in fp16 (host-side
  layout marshalling only; all model arithmetic runs on device). Host
  layouts are arranged so every input DMA is fully contiguous (8KB/
  partition lines) -- 256B strided descriptors starved the PE during the
  first matmul (all dynamic DMAs share one SP HW-DGE queue, ~220 GB/s).
- Output is written d-major ([BS, D*OUT], host transposes): the last step's
  V writes are then contiguous DVE ops; d-interleaved strided writes
  measured ~2.3x slower and serialized the end-of-kernel drain.
- Last step runs b-tile order [3,0,1,2] per chunk-1 so the final tile's
  normalize overlaps other tiles' matmuls; u-staging copies run on DVE
  (the drain serializes on the ACT chain); the final tile is 4x NW=256
  units, last two reading PSUM directly (banks need not free early).
- Tile pre-splits every Matmult into Ldweights+Matmult; a BIR-JSON post-pass
  drops Ldweights that reload the identical stationary (the d-loop reuses
  each K' tile 4x), and bacc's act-table pass is disabled in favor of
  walrus lower_act (bacc's greedy alternated two tables 289x per kernel).
Measured: ~1.036 ms HW exec (from 1.064 ms baseline), rel err 3.45e-3
(gate 2e-2); PE union busy ~97% of span; fp16 streaming floor for the
4608-matmul schedule is ~994 us + ~5 us HAM warm-up + ~5 us epilogue.
fp8 was evaluated and rejected: quantization error decays only ~0.82x/
step through the normalize, so even one e4m3 step exceeds the gate.
"""
import contextlib
import ctypes
import os
import sys
import types

import numpy as np

B, IN, OUT, D = 8192, 1024, 1024, 4
STEPS = 8
NCORES = 8
BS = B // NCORES      # batch shard per core = 1024
CH = 512              # on-chip batch chunk (2 chunks, processed serially)
NCH = BS // CH
P = 128
NT = OUT // P         # 8 partition tiles

_SO_PATH = "/opt/axon/libaxon_pjrt.so"


# ---------------------------------------------------------------- plumbing
def _ntff_profile_via_ctypes(so_path):
    try:
        lib = ctypes.CDLL(so_path)
    except OSError:
        return None
    if not hasattr(lib, "axon_start_nrt_profile"):
        return None
    lib.axon_start_nrt_profile.argtypes = [ctypes.POINTER(ctypes.c_int64), ctypes.c_size_t]
    lib.axon_start_nrt_profile.restype = ctypes.c_int64
    lib.axon_stop_nrt_profile.argtypes = [ctypes.c_char_p]
    lib.axon_stop_nrt_profile.restype = ctypes.c_int64

    @contextlib.contextmanager
    def _hook(output_dir, device_ids):
        import jax

        jax.devices()
        if device_ids:
            ids = (ctypes.c_int64 * len(device_ids))(*device_ids)
            rc = lib.axon_start_nrt_profile(ids, len(device_ids))
        else:
            rc = lib.axon_start_nrt_profile(None, 0)
        if rc != 0:
            raise RuntimeError(f"axon_start_nrt_profile rc={rc}")
        try:
            yield
        finally:
            n = lib.axon_stop_nrt_profile(str(output_dir).encode())
            print(f"profile: {n} file(s) written to {output_dir}", file=sys.stderr)

    return _hook


def _install_hook_shim():
    if "antenv.axon_hooks" in sys.modules:
        return
    try:
        import antenv
    except ImportError:
        return
    mod = types.ModuleType("antenv.axon_hooks")
    _state = {"hook": _ntff_profile_via_ctypes(_SO_PATH)}
    mod.set_axon_ntff_profile_hook = lambda h: _state.__setitem__("hook", h)
    mod.get_axon_ntff_profile_hook = lambda: _state["hook"]
    sys.modules["antenv.axon_hooks"] = mod
    antenv.axon_hooks = mod


def _patch_ldw_opt():
    import concourse.bass_utils as bu

    if os.environ.get("KERNEL_FUSE") != "1":
        return
    if getattr(bu, "_ldw_patched", False):
        return
    orig = bu.run_command

    def patched(argv, **kwargs):
        argv = [
            a.replace("--enable-ldw-opt=false", "--enable-ldw-opt=true")
            if isinstance(a, str)
            else a
            for a in argv
        ]
        return orig(argv, **kwargs)

    bu.run_command = patched
    bu._ldw_patched = True


def _dedup_ldweights_json(nc):
    """Drop Ldweights that reload the exact weights already resident
    (identical operand AP as previous Ldweights, only Matmults between)."""
    import orjson

    orig = nc.to_json_bytes

    def patched():
        bir = orjson.loads(orig())
        n = 0
        for f in bir.get("functions", []):
            for blk in f.get("blocks", []):
                insts = blk.get("instructions")
                if not insts:
                    continue
                keep = []
                last_w = None
                for ins in insts:
                    op = ins.get("opcode")
                    if op == "Ldweights":
                        si = ins.get("sync_info") or {}
                        wts = si.get("on_wait") or []
                        key = orjson.dumps(ins.get("ins"))
                        if key == last_w and not wts:
                            n += 1
                            continue
                        last_w = key
                        keep.append(ins)
                    elif op == "Matmult":
                        keep.append(ins)
                    else:
                        last_w = None
                        keep.append(ins)
                blk["instructions"] = keep
        return orjson.dumps(bir)

    nc.to_json_bytes = patched
    return nc


def _fuse_ldweights_json(nc):
    import orjson

    orig = nc.to_json_bytes

    def patched():
        bir = orjson.loads(orig())
        for f in bir.get("functions", []):
            for blk in f.get("blocks", []):
                insts = blk.get("instructions")
                if not insts:
                    continue
                keep = []
                for ins in insts:
                    if ins.get("opcode") == "Ldweights":
                        si = ins.get("sync_info") or {}
                        w = si.get("on_wait") or []
                        assert not (si.get("on_update") or []), ins["name"]
                        if w:
                            keep.append({
                                "opcode": "NoOp",
                                "name": ins["name"],
                                "engine": ins.get("engine", "PE"),
                                "ins": [],
                                "outs": [],
                                "sync_info": {"on_wait": w, "on_update": []},
                            })
                        continue
                    keep.append(ins)
                blk["instructions"] = keep
        return orjson.dumps(bir)

    nc.to_json_bytes = patched
    return nc


# ---------------------------------------------------------------- builder
def _build(omega_zero):
    import concourse.bacc as bacc
    import concourse.mybir as mybir
    from concourse import tile

    A = mybir.ActivationFunctionType
    Op = mybir.AluOpType
    f32 = mybir.dt.float32
    bf16 = mybir.dt.float16  # fp16: 8x finer mantissa than bf16, same PE speed

    class BaccNoSplit(bacc.Bacc):
        def move_matmul_waits_to_ldweights(self):
            return

        def insert_act_table_loads(self):
            # walrus lower_act picks act-func sets globally (bacc's greedy
            # alternates natural_log/exp_and_others per normalize unit,
            # 289 table reloads)
            return

    nc = BaccNoSplit(None, target_bir_lowering=False)

    if os.environ.get("KERNEL_KEEPQ") != "1":
        # qPoolDynamic (SWDGE) is unused -- memset is an engine op, and all
        # DMAs go through the two HWDGE queues. Dropping it shrinks the
        # end-of-NEFF teardown, which waits per allocated physical queue.
        nc.m.queues = [q for q in nc.m.queues if q.name != "qPoolDynamic"]

    # xh[c*2+hx, p, tl*CH+b] = x[c*CH+b, (4hx+tl)*P+p]; wh[ot*4+qh, p,
    # itl*D*P+d*P+o] = W_in[(2qh+itl)*P+p, ot*P+o, d] -- both DMA as fully
    # contiguous blocks (4KB/2KB per-partition lines) instead of 256B
    # strided descriptors. W rides in 0.25MB quarters (bufs=16) and x in
    # 0.5MB halves: the 16 shared DMA engines round-robin across all
    # outstanding descriptors, so smaller descriptors complete sooner and
    # the PE's per-tile need times are met (1MB tiles starved the PE at
    # ot=3..6 and re-throttled HAM).
    xh = nc.declare_dram_parameter("xh", [NCH * 2, P, (NT // 2) * CH], mybir.dt.float16, isOutput=False)
    wh = nc.declare_dram_parameter("wh", [NT * 2, P, (NT // 2) * D * P], mybir.dt.float16, isOutput=False)
    ct = nc.declare_dram_parameter("ct", [OUT, OUT], mybir.dt.float16, isOutput=False)
    if not omega_zero:
        om = nc.declare_dram_parameter("om", [OUT, D], f32, isOutput=False)
        omr = nc.declare_dram_parameter("omr", [D, OUT], f32, isOutput=False)
    eye_in = nc.declare_dram_parameter("eye_in", [P, P], mybir.dt.float16, isOutput=False)
    # d-major output layout [BS, D*OUT]: every DVE write in the last step
    # is contiguous (strided d-interleave writes measured ~2.3x slower) and
    # the DMA still moves 2KB runs. Host transposes to [BS, OUT, D].
    out = nc.declare_dram_parameter("out", [BS, D * OUT], f32, isOutput=True)

    with tile.TileContext(nc) as tc, contextlib.ExitStack() as ctx:
        const = ctx.enter_context(tc.tile_pool(name="const", bufs=1))
        pool = ctx.enter_context(tc.tile_pool(name="pool", bufs=1))
        psum = ctx.enter_context(tc.tile_pool(name="psum", bufs=1, space="PSUM"))

        omc = []
        omrb = []
        ktb = []
        ones1_box = []
        wb_gate = []  # chunk-0 W-half tiles; emit_preamble gates ktf on one

        def emit_preamble():
            # constants / K' = tanh(ct)^T rows + I. Emitted AFTER chunk 0's
            # first matmul so the PE starts as soon as xt/wb land. All
            # preamble DMAs ride the Activation HWDGE queue so they never
            # contend with the W stream on the SP queue.
            eyeb = const.tile([P, P], bf16, name="eyeb", tag="eyeb")
            nc.scalar.dma_start(eyeb[:], eye_in[:])

            if not omega_zero:
                for t in range(NT):  # omega columns per o-tile: [128, D] fp32
                    o = const.tile([P, D], f32, name=f"omc{t}", tag=f"omc{t}")
                    nc.sync.dma_start(o[:], om[t * P:(t + 1) * P, :])
                    omc.append(o)
                for d in range(D):  # omega rows per d: [1, OUT] fp16
                    of = const.tile([1, OUT], f32, name=f"omrf{d}", tag=f"omrf{d}")
                    nc.sync.dma_start(of[:], omr[d:d + 1, :])
                    ob = const.tile([1, OUT], bf16, name=f"omrb{d}", tag=f"omrb{d}")
                    nc.vector.tensor_copy(ob[:], of[:])
                    omrb.append(ob)
                ones1 = const.tile([1, P], bf16, name="ones1", tag="ones1")
                nc.gpsimd.memset(ones1[:], 1.0)
                ones1_box.append(ones1)

            for j in range(NT):  # K'^T tiles: [128(j), OUT(i)] fp16
                # ktf rides the SP queue, emitted AFTER all 16 W-half DMAs:
                # its issues queue up behind the MM-gated W issues, so the
                # 2MB of coupling data only moves at ~40-55us -- leaving the
                # early window's full bandwidth to W (W tiles arriving
                # just-late starved the PE at ot2..6 and re-throttled HAM).
                # K' is still ready well before step 1 needs it at ~66us.
                # bufs=8: every ktf DMA issue is ungated by Tanh progress (a
                # gated issue would block later compute in an engine stream).
                kf = pool.tile([P, OUT], bf16, name=f"ktf{j}", tag="ktf", bufs=8)
                if wb_gate:
                    # WAW-gate the ktf DMA on W-half #9 having LANDED (a
                    # 1-element DVE write into kf forces the ordering): the
                    # scheduler otherwise hoists the dep-free ktf issues
                    # into the early window where their 2MB starves W.
                    nc.vector.tensor_copy(kf[:, 0:1], wb_gate[9][:, 0:1])
                nc.sync.dma_start(kf[:], ct[j * P:(j + 1) * P, :])
                kb = const.tile([P, OUT], bf16, name=f"ktb{j}", tag=f"ktb{j}")
                nc.scalar.activation(kb[:], kf[:], A.Tanh)
                nc.vector.tensor_tensor(
                    kb[:, j * P:(j + 1) * P], kb[:, j * P:(j + 1) * P], eyeb[:],
                    op=Op.add,
                )
                ktb.append(kb)

        # v planes: tag per (d, j), 2 bufs (generation ping-pong)
        def v_tile(d, j, s):
            return const.tile([P, CH], bf16, name=f"v_s{s}_d{d}_j{j}",
                             tag=f"v{d}_{j}", bufs=2)

        def normalize_unit(ps, bias_aps, vout, n, label):
            """ps: 4 psum APs [P,n] (separate per-d tiles: Tile's WAR
            tracking is whole-tile, so per-d tiles give the staggered
            bank release the next-next unit's d-ordered matmuls need --
            a merged 4-bank tile measured +0.7us PE stall per unit).
            bias_aps: 4 per-part scalars or None; vout(d, inv) -> emits
            the final scaled write for plane d."""
            q = [pool.tile([P, n], bf16, name=f"q{d}_{label}", tag=f"q{d}", bufs=2)
                 for d in range(D)]
            for d in range(D):
                if bias_aps is None:
                    nc.scalar.activation(q[d][:], ps[d], A.Square)
                else:
                    nc.scalar.activation(q[d][:], ps[d], A.Square, bias=bias_aps[d])
            s01 = pool.tile([P, n], bf16, name=f"s01_{label}", tag="s01", bufs=2)
            s23 = pool.tile([P, n], bf16, name=f"s23_{label}", tag="s23", bufs=2)
            ssum = pool.tile([P, n], bf16, name=f"ss_{label}", tag="ss", bufs=2)
            nc.vector.tensor_tensor(s01[:], q[0][:], q[1][:], op=Op.add)
            nc.vector.tensor_tensor(s23[:], q[2][:], q[3][:], op=Op.add)
            nc.vector.tensor_tensor(ssum[:], s01[:], s23[:], op=Op.add)
            lns = pool.tile([P, n], f32, name=f"ln_{label}", tag="lns", bufs=2)
            nc.scalar.activation(lns[:], ssum[:], A.Ln)
            inv = pool.tile([P, n], f32, name=f"inv_{label}", tag="inv", bufs=2)
            nc.scalar.activation(inv[:], lns[:], A.Exp, scale=-0.5)
            for d in range(D):
                vout(d, inv)

        def first_matmul(c):
            # v0 = l2norm(x @ W_in). x rides the Act queue in two halves
            # (needed first, lands in parallel with the W stream on the SP
            # queue; the first matmuls only need half 0).
            # bufs=2: chunk 1's halves WAR-wait on chunk 0's being fully
            # consumed (~63us) -- with bufs=4 the scheduler hoisted chunk
            # 1's 1MB into the bandwidth-critical first 15us.
            xth = []
            for hx in range(2):
                t = pool.tile([P, (NT // 2) * CH], bf16,
                              name=f"xt{c}_{hx}", tag="xt", bufs=2)
                nc.scalar.dma_start(t[:], xh[c * 2 + hx])
                xth.append(t)

            HIT = NT // 2  # it-tiles per W half
            vcur = {}
            for ot in range(NT):
                ps = [psum.tile([P, CH], f32, name=f"ps0_{c}_{ot}_{d}",
                                tag=f"ps{d}", bufs=2) for d in range(D)]
                for h in range(2):
                    wbh = pool.tile([P, HIT * D * P], bf16,
                                    name=f"wb{c}_{ot}_{h}", tag="wb", bufs=8)
                    if c == 0 and len(wb_gate) >= 2:
                        # chain-gate chunk 0's W stream at distance 2: the
                        # DMA queue round-robins 4KB packets fairly across
                        # ALL in-flight descriptors, so K outstanding W
                        # tiles each crawl at BW/K and the first ones land
                        # late (PE stalled ~2us at each of ot2..6). With <=2
                        # W descriptors in flight each completes in ~2.5us,
                        # ahead of the PE's 3.45us/tile consumption.
                        nc.vector.tensor_copy(wbh[:, 0:1],
                                              wb_gate[len(wb_gate) - 2][:, 0:1])
                    nc.sync.dma_start(wbh[:], wh[ot * 2 + h])
                    if c == 0:
                        wb_gate.append(wbh)
                    for itl in range(HIT):
                        it = h * HIT + itl
                        for d in range(D):
                            nc.tensor.matmul(
                                ps[d][:],
                                wbh[:, itl * D * P + d * P:itl * D * P + (d + 1) * P],
                                xth[it // 4][:, (it % 4) * CH:(it % 4 + 1) * CH],
                                start=(it == 0), stop=(it == NT - 1),
                            )

                def vout0(d, inv, _ps=ps, _ot=ot, _c=c):
                    vt = v_tile(d, _ot, 0)
                    vcur.setdefault(d, {})[_ot] = vt
                    nc.vector.tensor_tensor(vt[:], _ps[d][:], inv[:], op=Op.mult)

                normalize_unit([p[:] for p in ps], None, vout0, CH, f"f{c}_{ot}")
            return vcur

        # PE warm-up: dummy matmuls on memset data fill the initial
        # input-DMA wait and lift the HAM clock gate to 2.4 GHz before the
        # first real matmul issues. Sized to bridge until the first weight
        # tile lands (~12.5us): a >3.4us PE-idle gap between warm-up and
        # the first real matmul re-throttles the clock to 1.2 GHz and the
        # whole first o-tile runs at half speed (seen as a HAM K=4/8 event
        # at ~15.6us in the trace). First ~8 run cold (~427ns), rest warm
        # (~214ns): 40 MMs end ~10.6us.
        wuw = pool.tile([P, P], bf16, name="wuw", tag="wuw")
        wux = pool.tile([P, 512], bf16, name="wux", tag="wux")
        nc.gpsimd.memset(wuw[:], 0.0)
        nc.gpsimd.memset(wux[:], 0.0)
        wups = psum.tile([P, 512], f32, name="wups", tag="ps0", bufs=2)
        for _ in range(28):
            nc.tensor.matmul(wups[:], wuw[:], wux[:], start=True, stop=True)

        # first_matmul first (higher scheduler priority for its ACT
        # normalize ops); the preamble's ktf DMA issues are dep-free so the
        # scheduler still hoists them into the Act engine's idle start.
        vcur_pending = {0: first_matmul(0)}
        emit_preamble()

        for c in range(NCH):
            vcur = vcur_pending.pop(c)
            # ---------------- steps 1..STEPS-1 (normal orientation) ------
            for s in range(1, STEPS):
                vnext = {}
                for it in range(NT):
                    ps = [psum.tile([P, CH], f32, name=f"ps{s}_{c}_{it}_{d}",
                                    tag=f"ps{d}", bufs=2) for d in range(D)]
                    for j in range(NT):
                        for d in range(D):
                            nc.tensor.matmul(
                                ps[d][:],
                                ktb[j][:, it * P:(it + 1) * P],
                                vcur[d][j][:],
                                start=(j == 0), stop=(j == NT - 1),
                            )
                    bias_aps = (None if omega_zero else
                                [omc[it][:, d:d + 1] for d in range(D)])

                    def vouts(d, inv, _ps=ps, _it=it, _s=s):
                        vt = v_tile(d, _it, _s)
                        vnext.setdefault(d, {})[_it] = vt
                        if omega_zero:
                            nc.vector.tensor_tensor(
                                vt[:], _ps[d][:], inv[:], op=Op.mult)
                        else:
                            nc.vector.scalar_tensor_tensor(
                                vt[:], _ps[d][:], omc[_it][:, d:d + 1], inv[:],
                                op0=Op.add, op1=Op.mult,
                            )

                    normalize_unit([p[:] for p in ps], bias_aps, vouts, CH,
                                   f"s{s}_{c}_{it}")
                vcur = vnext

            # chunk c+1's first matmul emitted here: its matmuls fill the
            # last-step drain stalls, and its v0 slots are free by now.
            if c + 1 < NCH:
                vcur_pending[c + 1] = first_matmul(c + 1)

            # ---------------- last step, transposed: u[b, i] --------------
            # final chunk: the final b-tile (bt=2) is split into quarter
            # units; its first two (staged) units run EARLY -- right after
            # bt=3 -- so only the two slim quarter units remain at the very
            # end, and their normalize is all the un-overlappable drain.
            # (seq entries: bt, col, NW, slim, last_unit)
            if c == NCH - 1:
                seq = ([(bt, col, 512, False, False)
                        for bt in (3, 0, 1) for col in (0, 512)] +
                       [(2, 0, 256, False, False), (2, 256, 256, False, False),
                        (2, 512, 256, False, False),
                        (2, 768, 128, True, False), (2, 896, 128, True, True)])
            else:
                seq = [(bt, col, 512, False, False)
                       for bt in range(CH // P) for col in (0, 512)]
            for ic, (bt, col, NW, slim, last_unit) in enumerate(seq):
                    ps = [psum.tile([P, NW], f32, name=f"psL_{c}_{bt}_{ic}_{d}",
                                    tag=f"ps{d}", bufs=2) for d in range(D)]
                    for j in range(NT):
                        for d in range(D):
                            nc.tensor.matmul(
                                ps[d][:],
                                vcur[d][j][:, bt * P:(bt + 1) * P],
                                ktb[j][:, col:col + NW],
                                start=(j == 0),
                                stop=(omega_zero and j == NT - 1),
                            )
                    if not omega_zero:
                        for d in range(D):
                            nc.tensor.matmul(
                                ps[d][:],
                                ones1_box[0][:],
                                omrb[d][:, col:col + NW],
                                start=False, stop=True,
                            )

                    if slim:
                        u = ps
                        nsrc = [p[:] for p in ps]
                    else:
                        # stage u out of PSUM early so the banks free in
                        # ~1.7us instead of being held through the V writes.
                        # On DVE, not ACT: the last step's ACT chain (4x
                        # Square + Ln + Exp per unit) is what the end-of-
                        # kernel drain serializes on. The WHOLE normalize
                        # (Squares included) reads u, so the copies are the
                        # only PSUM readers -- otherwise the next-next
                        # unit's matmuls wait on the (late, DVE-gated) ACT
                        # Squares for bank release.
                        u = [pool.tile([P, NW], f32,
                                       name=f"u{d}_L{c}_{bt}_{ic}",
                                       tag=f"u{d}", bufs=2) for d in range(D)]
                        for d in range(D):
                            nc.vector.tensor_copy(u[d][:], ps[d][:])
                        nsrc = [t[:] for t in u]

                    if last_unit:
                        # split the very last output DMA per d-pair so the
                        # first half's transfer overlaps the second half's
                        # multiplies.
                        Vp = [pool.tile([P, NW * 2], f32,
                                        name=f"Vp{pr}_{c}_{bt}_{ic}",
                                        tag=f"bigh{pr}", bufs=1)
                              for pr in range(2)]

                        def voutL(d, inv, _Vp=Vp, _u=u, _NW=NW):
                            nc.vector.tensor_tensor(
                                _Vp[d // 2][:, (d % 2) * _NW:(d % 2 + 1) * _NW],
                                _u[d][:], inv[:], op=Op.mult)

                        normalize_unit(nsrc, None, voutL, NW,
                                       f"L{c}_{bt}_{ic}")
                        obase = (out[(c * CH + bt * P):(c * CH + (bt + 1) * P), :]
                                 .rearrange("r (d o) -> r d o", d=D))
                        for pr in range(2):
                            nc.sync.dma_start(
                                obase[:, 2 * pr:2 * pr + 2, col:col + NW],
                                Vp[pr][:].rearrange("p (d n) -> p d n", d=2),
                            )
                    else:
                        V = pool.tile([P, NW * D], f32, name=f"V{c}_{bt}_{ic}",
                                      tag="big", bufs=2)

                        def voutL(d, inv, _V=V, _u=u, _NW=NW):
                            nc.vector.tensor_tensor(
                                _V[:, d * _NW:(d + 1) * _NW], _u[d][:], inv[:],
                                op=Op.mult,
                            )

                        normalize_unit(nsrc, None, voutL, NW,
                                       f"L{c}_{bt}_{ic}")
                        nc.sync.dma_start(
                            out[(c * CH + bt * P):(c * CH + (bt + 1) * P), :]
                            .rearrange("r (d o) -> r d o", d=D)[:, :, col:col + NW],
                            V[:].rearrange("p (d n) -> p d n", d=D),
                        )

    nc.finalize()
    if os.environ.get("KERNEL_FUSE") == "1":
        _fuse_ldweights_json(nc)
    elif os.environ.get("KERNEL_NODEDUP") != "1":
        _dedup_ldweights_json(nc)
    return nc


_CACHED = {}


def kernel(x, W_in, omega, coupling):
    _install_hook_shim()
    _patch_ldw_opt()
    from concourse.bass_utils import run_bass_kernel_spmd

    x = np.ascontiguousarray(np.asarray(x, dtype=np.float32))
    W_in = np.asarray(W_in, dtype=np.float32)
    omega = np.ascontiguousarray(np.asarray(omega, dtype=np.float32))
    coupling = np.asarray(coupling, dtype=np.float32)

    # wh[ot*2+h, p, itl*D*P + d*P + o] = W_in[(4h+itl)*P+p, ot*P+o, d]
    wt = W_in.transpose(2, 0, 1).astype(np.float16)                # [D, IN, OUT]
    wh_host = np.ascontiguousarray(
        wt.reshape(D, NT, P, NT, P)          # [d, it, p, ot, o]
        .transpose(3, 1, 2, 0, 4)            # [ot, it, p, d, o]
        .reshape(NT, 2, NT // 2, P, D, P)    # [ot, h, itl, p, d, o]
        .transpose(0, 1, 3, 2, 4, 5)         # [ot, h, p, itl, d, o]
        .reshape(NT * 2, P, (NT // 2) * D * P))
    ct_host = np.ascontiguousarray(coupling.T.astype(np.float16))  # [OUT, OUT]
    omr_host = np.ascontiguousarray(omega.T)                       # [D, OUT]
    eye_host = np.eye(P, dtype=np.float16)

    omega_zero = not np.any(omega)
    key = ("nc", omega_zero)
    if key not in _CACHED:
        _CACHED[key] = _build(omega_zero)
    nc = _CACHED[key]

    in_maps = []
    for core in range(NCORES):
        xs = x[core * BS:(core + 1) * BS, :]
        # xh[c*2+hx, p, tl*CH + b] = xs[c*CH+b, (4hx+tl)*P+p]
        xh_host = np.ascontiguousarray(
            xs.T.astype(np.float16)
            .reshape(2, NT // 2, P, NCH, CH)     # [hx, tl, p, c, b]
            .transpose(3, 0, 2, 1, 4)            # [c, hx, p, tl, b]
            .reshape(NCH * 2, P, (NT // 2) * CH))
        im = {
            "xh": xh_host,
            "wh": wh_host,
            "ct": ct_host,
            "eye_in": eye_host,
        }
        if not omega_zero:
            im["om"] = omega
            im["omr"] = omr_host
        in_maps.append(im)

    trace = os.environ.get("KERNEL_TRACE") == "1"
    res = run_bass_kernel_spmd(nc, in_maps, core_ids=list(range(NCORES)), trace=trace)
    if trace and res.exec_time_ns:
        print(f"HW exec time: {res.exec_time_ns} ns")
        _CACHED["exec_time_ns"] = res.exec_time_ns
        _CACHED["results"] = res

    outs = [res.results[i]["out"].reshape(BS, D, OUT).transpose(0, 2, 1)
            for i in range(NCORES)]
    return np.ascontiguousarray(np.concatenate(outs, axis=0))



# revision 43
# speedup vs baseline: 1.0057x; 1.0057x over previous
"""AKOrN layer on 8 TRN2 NeuronCores, data-parallel over batch.

reference: v = l2norm_d(x @ W_in); K = tanh(coupling);
           8x: v = l2norm_d(v + K @ v + omega); return v [B, OUT, D]

Implementation notes:
- Data-parallel: batch 8192 -> 1024 rows per core; W_in/coupling/omega
  replicated. No collectives.
- K' = tanh(coupling) + I folds the "+ v" into the step matmul, so each step
  is pure matmul work plus a PSUM-side normalize.
- v lives on-chip as 4 per-d planes [OUT(part), batch(free)] in fp16 (8x
  finer mantissa than bf16 at identical PE speed; bf16 landed at rel err
  2.4e-2, fp16 at 3.3e-3). Batch is processed in 2 sequential 512-column
  chunks (SBUF fit for the double-buffered v generations).
- Step: 8 j-tiles x 4 d matmuls accumulate K'^T @ v_d into 4 PSUM banks
  (2 normalize units in flight = all 8 banks), then:
    q_d = Square(psum_d + omega_d)        (ACT, per-partition bias, fp16 out)
    s   = q0+q1+q2+q3                     (DVE fp16, 2x mode)
    inv = Exp(-0.5 * Ln(s))               (ACT, == rsqrt(s), one act table)
    v'_d = (psum_d + omega_d) * inv       (DVE scalar_tensor_tensor -> fp16)
- Last step runs transposed (stationary = v-slice, moving = K'^T rows) to
  produce [batch, OUT] so the d-interleave + output DMA is contiguous;
  omega enters there via a K=1 ones-row matmul (skipped when omega == 0).
  u is staged out of PSUM by ACT copies so banks free early.
- chunk1's first matmul is emitted between chunk0's steps and last step:
  its matmuls fill the last-step PSUM-drain stalls.
- x^T and W_in are uploaded pre-transposed/de-interleaved in fp16 (host-side
  layout marshalling only; all model arithmetic runs on device). Host
  layouts are arranged so every input DMA is fully contiguous (8KB/
  partition lines) -- 256B strided descriptors starved the PE during the
  first matmul (all dynamic DMAs share one SP HW-DGE queue, ~220 GB/s).
- Output is written d-major ([BS, D*OUT], host transposes): the last step's
  V writes are then contiguous DVE ops; d-interleaved strided writes
  measured ~2.3x slower and serialized the end-of-kernel drain.
- Last step runs b-tile order [3,0,1,2] per chunk-1 so the final tile's
  normalize overlaps other tiles' matmuls; u-staging copies run on DVE and
  the whole normalize reads the staged u (banks free right after the
  copies); the final tile is 3x NW=256 staged + 2x NW=128 slim units, and
  the very last unit's output DMA is split per d-pair so the transfer
  overlaps the remaining multiplies.
- Input staging: W streams on the SP HWDGE queue in it-major 0.5MB halves
  (bufs=8); x rides the Act HWDGE queue in halves; the 2MB coupling load is
  WAW-gated on W-half #9 landing so it cannot steal early bandwidth (the
  16 shared DMA engines round-robin packets fairly across ALL in-flight
  descriptors -- K outstanding tiles each get BW/K, so extra early
  descriptors make the first W tiles late, stall the PE at ot2..6, and
  re-throttle HAM). ktf bufs=8 keeps every coupling DMA issue ungated (a
  gated DMA-issue blocks later compute ops in that engine's stream).
- qPoolDynamic (unused SWDGE queue set) is dropped from nc.m.queues
  (50 -> 34 allocated queues); the end-of-NEFF teardown (~9us of
  per-semaphore waits on every engine) is otherwise a fixed cost.
- Tile pre-splits every Matmult into Ldweights+Matmult; a BIR-JSON post-pass
  drops Ldweights that reload the identical stationary (the d-loop reuses
  each K' tile 4x), and bacc's act-table pass is disabled in favor of
  walrus lower_act (bacc's greedy alternated two tables 289x per kernel).
Measured: ~1.034-1.037 ms HW exec (run-to-run spread; occasional runs hit
a chip-level P0 power downclock to 2.0 GHz and read ~1.2 ms), rel err
3.45e-3 (gate 2e-2); PE busy ~97%; fp16 streaming floor for the ~4600-MM
schedule is ~994 us + ~10 us prologue/HAM warm-up + ~17 us drain+teardown.
fp8/DoubleRow was evaluated and rejected: the dynamics AMPLIFY injected
quantization error ~1.2x/step (numpy+ml_dtypes sim: one e4m3 step in the
last position alone gives 4.1e-2 > gate; all-fp16 sim 2.8e-3 matches HW).
Keep per-d PSUM tiles: Tile's WAR tracking is whole-tile, so a merged
4-bank tile makes unit k+2's matmuls wait the last d-mul (+0.7us/unit).
"""
import contextlib
import ctypes
import os
import sys
import types

import numpy as np

B, IN, OUT, D = 8192, 1024, 1024, 4
STEPS = 8
NCORES = 8
BS = B // NCORES      # batch shard per core = 1024
CH = 512              # on-chip batch chunk (2 chunks, processed serially)
NCH = BS // CH
P = 128
NT = OUT // P         # 8 partition tiles

_SO_PATH = "/opt/axon/libaxon_pjrt.so"


# ---------------------------------------------------------------- plumbing
def _ntff_profile_via_ctypes(so_path):
    try:
        lib = ctypes.CDLL(so_path)
    except OSError:
        return None
    if not hasattr(lib, "axon_start_nrt_profile"):
        return None
    lib.axon_start_nrt_profile.argtypes = [ctypes.POINTER(ctypes.c_int64), ctypes.c_size_t]
    lib.axon_start_nrt_profile.restype = ctypes.c_int64
    lib.axon_stop_nrt_profile.argtypes = [ctypes.c_char_p]
    lib.axon_stop_nrt_profile.restype = ctypes.c_int64

    @contextlib.contextmanager
    def _hook(output_dir, device_ids):
        import jax

        jax.devices()
        if device_ids:
            ids = (ctypes.c_int64 * len(device_ids))(*device_ids)
            rc = lib.axon_start_nrt_profile(ids, len(device_ids))
        else:
            rc = lib.axon_start_nrt_profile(None, 0)
        if rc != 0:
            raise RuntimeError(f"axon_start_nrt_profile rc={rc}")
        try:
            yield
        finally:
            n = lib.axon_stop_nrt_profile(str(output_dir).encode())
            print(f"profile: {n} file(s) written to {output_dir}", file=sys.stderr)

    return _hook


def _install_hook_shim():
    if "antenv.axon_hooks" in sys.modules:
        return
    try:
        import antenv
    except ImportError:
        return
    mod = types.ModuleType("antenv.axon_hooks")
    _state = {"hook": _ntff_profile_via_ctypes(_SO_PATH)}
    mod.set_axon_ntff_profile_hook = lambda h: _state.__setitem__("hook", h)
    mod.get_axon_ntff_profile_hook = lambda: _state["hook"]
    sys.modules["antenv.axon_hooks"] = mod
    antenv.axon_hooks = mod


def _patch_ldw_opt():
    import concourse.bass_utils as bu

    if os.environ.get("KERNEL_FUSE") != "1":
        return
    if getattr(bu, "_ldw_patched", False):
        return
    orig = bu.run_command

    def patched(argv, **kwargs):
        argv = [
            a.replace("--enable-ldw-opt=false", "--enable-ldw-opt=true")
            if isinstance(a, str)
            else a
            for a in argv
        ]
        return orig(argv, **kwargs)

    bu.run_command = patched
    bu._ldw_patched = True


def _dedup_ldweights_json(nc):
    """Drop Ldweights that reload the exact weights already resident
    (identical operand AP as previous Ldweights, only Matmults between)."""
    import orjson

    orig = nc.to_json_bytes

    def patched():
        bir = orjson.loads(orig())
        n = 0
        for f in bir.get("functions", []):
            for blk in f.get("blocks", []):
                insts = blk.get("instructions")
                if not insts:
                    continue
                keep = []
                last_w = None
                for ins in insts:
                    op = ins.get("opcode")
                    if op == "Ldweights":
                        si = ins.get("sync_info") or {}
                        wts = si.get("on_wait") or []
                        key = orjson.dumps(ins.get("ins"))
                        if key == last_w and not wts:
                            n += 1
                            continue
                        last_w = key
                        keep.append(ins)
                    elif op == "Matmult":
                        keep.append(ins)
                    else:
                        last_w = None
                        keep.append(ins)
                blk["instructions"] = keep
        return orjson.dumps(bir)

    nc.to_json_bytes = patched
    return nc


def _fuse_ldweights_json(nc):
    import orjson

    orig = nc.to_json_bytes

    def patched():
        bir = orjson.loads(orig())
        for f in bir.get("functions", []):
            for blk in f.get("blocks", []):
                insts = blk.get("instructions")
                if not insts:
                    continue
                keep = []
                for ins in insts:
                    if ins.get("opcode") == "Ldweights":
                        si = ins.get("sync_info") or {}
                        w = si.get("on_wait") or []
                        assert not (si.get("on_update") or []), ins["name"]
                        if w:
                            keep.append({
                                "opcode": "NoOp",
                                "name": ins["name"],
                                "engine": ins.get("engine", "PE"),
                                "ins": [],
                                "outs": [],
                                "sync_info": {"on_wait": w, "on_update": []},
                            })
                        continue
                    keep.append(ins)
                blk["instructions"] = keep
        return orjson.dumps(bir)

    nc.to_json_bytes = patched
    return nc


# ---------------------------------------------------------------- builder
def _build(omega_zero):
    import concourse.bacc as bacc
    import concourse.mybir as mybir
    from concourse import tile

    A = mybir.ActivationFunctionType
    Op = mybir.AluOpType
    f32 = mybir.dt.float32
    bf16 = mybir.dt.float16  # fp16: 8x finer mantissa than bf16, same PE speed

    class BaccNoSplit(bacc.Bacc):
        def move_matmul_waits_to_ldweights(self):
            return

        def insert_act_table_loads(self):
            # walrus lower_act picks act-func sets globally (bacc's greedy
            # alternates natural_log/exp_and_others per normalize unit,
            # 289 table reloads)
            return

    nc = BaccNoSplit(None, target_bir_lowering=False)

    if os.environ.get("KERNEL_KEEPQ") != "1":
        # qPoolDynamic (SWDGE) is unused -- memset is an engine op, and all
        # DMAs go through the two HWDGE queues. Dropping it shrinks the
        # end-of-NEFF teardown, which waits per allocated physical queue.
        nc.m.queues = [q for q in nc.m.queues if q.name != "qPoolDynamic"]

    # xh[c*2+hx, p, tl*CH+b] = x[c*CH+b, (4hx+tl)*P+p]; wh[ot*4+qh, p,
    # itl*D*P+d*P+o] = W_in[(2qh+itl)*P+p, ot*P+o, d] -- both DMA as fully
    # contiguous blocks (4KB/2KB per-partition lines) instead of 256B
    # strided descriptors. W rides in 0.25MB quarters (bufs=16) and x in
    # 0.5MB halves: the 16 shared DMA engines round-robin across all
    # outstanding descriptors, so smaller descriptors complete sooner and
    # the PE's per-tile need times are met (1MB tiles starved the PE at
    # ot=3..6 and re-throttled HAM).
    xh = nc.declare_dram_parameter("xh", [NCH * 2, P, (NT // 2) * CH], mybir.dt.float16, isOutput=False)
    wh = nc.declare_dram_parameter("wh", [NT * 2, P, (NT // 2) * D * P], mybir.dt.float16, isOutput=False)
    ct = nc.declare_dram_parameter("ct", [OUT, OUT], mybir.dt.float16, isOutput=False)
    if not omega_zero:
        om = nc.declare_dram_parameter("om", [OUT, D], f32, isOutput=False)
        omr = nc.declare_dram_parameter("omr", [D, OUT], f32, isOutput=False)
    eye_in = nc.declare_dram_parameter("eye_in", [P, P], mybir.dt.float16, isOutput=False)
    # d-major output layout [BS, D*OUT]: every DVE write in the last step
    # is contiguous (strided d-interleave writes measured ~2.3x slower) and
    # the DMA still moves 2KB runs. Host transposes to [BS, OUT, D].
    out = nc.declare_dram_parameter("out", [BS, D * OUT], f32, isOutput=True)

    with tile.TileContext(nc) as tc, contextlib.ExitStack() as ctx:
        const = ctx.enter_context(tc.tile_pool(name="const", bufs=1))
        pool = ctx.enter_context(tc.tile_pool(name="pool", bufs=1))
        psum = ctx.enter_context(tc.tile_pool(name="psum", bufs=1, space="PSUM"))

        omc = []
        omrb = []
        ktb = []
        ones1_box = []
        wb_gate = []  # chunk-0 W-half tiles; emit_preamble gates ktf on one

        def emit_preamble():
            # constants / K' = tanh(ct)^T rows + I. Emitted AFTER chunk 0's
            # first matmul so the PE starts as soon as xt/wb land. All
            # preamble DMAs ride the Activation HWDGE queue so they never
            # contend with the W stream on the SP queue.
            eyeb = const.tile([P, P], bf16, name="eyeb", tag="eyeb")
            nc.scalar.dma_start(eyeb[:], eye_in[:])

            if not omega_zero:
                for t in range(NT):  # omega columns per o-tile: [128, D] fp32
                    o = const.tile([P, D], f32, name=f"omc{t}", tag=f"omc{t}")
                    nc.sync.dma_start(o[:], om[t * P:(t + 1) * P, :])
                    omc.append(o)
                for d in range(D):  # omega rows per d: [1, OUT] fp16
                    of = const.tile([1, OUT], f32, name=f"omrf{d}", tag=f"omrf{d}")
                    nc.sync.dma_start(of[:], omr[d:d + 1, :])
                    ob = const.tile([1, OUT], bf16, name=f"omrb{d}", tag=f"omrb{d}")
                    nc.vector.tensor_copy(ob[:], of[:])
                    omrb.append(ob)
                ones1 = const.tile([1, P], bf16, name="ones1", tag="ones1")
                nc.gpsimd.memset(ones1[:], 1.0)
                ones1_box.append(ones1)

            for j in range(NT):  # K'^T tiles: [128(j), OUT(i)] fp16
                # ktf rides the SP queue, emitted AFTER all 16 W-half DMAs:
                # its issues queue up behind the MM-gated W issues, so the
                # 2MB of coupling data only moves at ~40-55us -- leaving the
                # early window's full bandwidth to W (W tiles arriving
                # just-late starved the PE at ot2..6 and re-throttled HAM).
                # K' is still ready well before step 1 needs it at ~66us.
                # bufs=8: every ktf DMA issue is ungated by Tanh progress (a
                # gated issue would block later compute in an engine stream).
                kf = pool.tile([P, OUT], bf16, name=f"ktf{j}", tag="ktf", bufs=8)
                if wb_gate:
                    # WAW-gate the ktf DMA on W-half #9 having LANDED (a
                    # 1-element DVE write into kf forces the ordering): the
                    # scheduler otherwise hoists the dep-free ktf issues
                    # into the early window where their 2MB starves W.
                    nc.vector.tensor_copy(kf[:, 0:1], wb_gate[9][:, 0:1])
                nc.sync.dma_start(kf[:], ct[j * P:(j + 1) * P, :])
                kb = const.tile([P, OUT], bf16, name=f"ktb{j}", tag=f"ktb{j}")
                nc.scalar.activation(kb[:], kf[:], A.Tanh)
                nc.vector.tensor_tensor(
                    kb[:, j * P:(j + 1) * P], kb[:, j * P:(j + 1) * P], eyeb[:],
                    op=Op.add,
                )
                ktb.append(kb)

        # v planes: tag per (d, j), 2 bufs (generation ping-pong)
        def v_tile(d, j, s):
            return const.tile([P, CH], bf16, name=f"v_s{s}_d{d}_j{j}",
                             tag=f"v{d}_{j}", bufs=2)

        def normalize_unit(ps, bias_aps, vout, n, label):
            """ps: 4 psum APs [P,n] (separate per-d tiles: Tile's WAR
            tracking is whole-tile, so per-d tiles give the staggered
            bank release the next-next unit's d-ordered matmuls need --
            a merged 4-bank tile measured +0.7us PE stall per unit).
            bias_aps: 4 per-part scalars or None; vout(d, inv) -> emits
            the final scaled write for plane d."""
            q = [pool.tile([P, n], bf16, name=f"q{d}_{label}", tag=f"q{d}", bufs=2)
                 for d in range(D)]
            for d in range(D):
                if bias_aps is None:
                    nc.scalar.activation(q[d][:], ps[d], A.Square)
                else:
                    nc.scalar.activation(q[d][:], ps[d], A.Square, bias=bias_aps[d])
            s01 = pool.tile([P, n], bf16, name=f"s01_{label}", tag="s01", bufs=2)
            s23 = pool.tile([P, n], bf16, name=f"s23_{label}", tag="s23", bufs=2)
            ssum = pool.tile([P, n], bf16, name=f"ss_{label}", tag="ss", bufs=2)
            nc.vector.tensor_tensor(s01[:], q[0][:], q[1][:], op=Op.add)
            nc.vector.tensor_tensor(s23[:], q[2][:], q[3][:], op=Op.add)
            nc.vector.tensor_tensor(ssum[:], s01[:], s23[:], op=Op.add)
            lns = pool.tile([P, n], f32, name=f"ln_{label}", tag="lns", bufs=2)
            nc.scalar.activation(lns[:], ssum[:], A.Ln)
            inv = pool.tile([P, n], f32, name=f"inv_{label}", tag="inv", bufs=2)
            nc.scalar.activation(inv[:], lns[:], A.Exp, scale=-0.5)
            for d in range(D):
                vout(d, inv)

        def first_matmul(c):
            # v0 = l2norm(x @ W_in). x rides the Act queue in two halves
            # (needed first, lands in parallel with the W stream on the SP
            # queue; the first matmuls only need half 0).
            # bufs=2: chunk 1's halves WAR-wait on chunk 0's being fully
            # consumed (~63us) -- with bufs=4 the scheduler hoisted chunk
            # 1's 1MB into the bandwidth-critical first 15us.
            xth = []
            for hx in range(2):
                t = pool.tile([P, (NT // 2) * CH], bf16,
                              name=f"xt{c}_{hx}", tag="xt", bufs=2)
                nc.scalar.dma_start(t[:], xh[c * 2 + hx])
                xth.append(t)

            HIT = NT // 2  # it-tiles per W half
            vcur = {}
            for ot in range(NT):
                ps = [psum.tile([P, CH], f32, name=f"ps0_{c}_{ot}_{d}",
                                tag=f"ps{d}", bufs=2) for d in range(D)]
                for h in range(2):
                    wbh = pool.tile([P, HIT * D * P], bf16,
                                    name=f"wb{c}_{ot}_{h}", tag="wb", bufs=8)
                    nc.sync.dma_start(wbh[:], wh[ot * 2 + h])
                    if c == 0:
                        wb_gate.append(wbh)
                    for itl in range(HIT):
                        it = h * HIT + itl
                        for d in range(D):
                            nc.tensor.matmul(
                                ps[d][:],
                                wbh[:, itl * D * P + d * P:itl * D * P + (d + 1) * P],
                                xth[it // 4][:, (it % 4) * CH:(it % 4 + 1) * CH],
                                start=(it == 0), stop=(it == NT - 1),
                            )

                def vout0(d, inv, _ps=ps, _ot=ot, _c=c):
                    vt = v_tile(d, _ot, 0)
                    vcur.setdefault(d, {})[_ot] = vt
                    nc.vector.tensor_tensor(vt[:], _ps[d][:], inv[:], op=Op.mult)

                normalize_unit([p[:] for p in ps], None, vout0, CH, f"f{c}_{ot}")
            return vcur

        # PE warm-up: dummy matmuls on memset data fill the initial
        # input-DMA wait and lift the HAM clock gate to 2.4 GHz before the
        # first real matmul issues. Sized to bridge until the first weight
        # tile lands (~12.5us): a >3.4us PE-idle gap between warm-up and
        # the first real matmul re-throttles the clock to 1.2 GHz and the
        # whole first o-tile runs at half speed (seen as a HAM K=4/8 event
        # at ~15.6us in the trace). First ~8 run cold (~427ns), rest warm
        # (~214ns): 40 MMs end ~10.6us.
        wuw = pool.tile([P, P], bf16, name="wuw", tag="wuw")
        wux = pool.tile([P, 512], bf16, name="wux", tag="wux")
        nc.gpsimd.memset(wuw[:], 0.0)
        nc.gpsimd.memset(wux[:], 0.0)
        wups = psum.tile([P, 512], f32, name="wups", tag="ps0", bufs=2)
        for _ in range(28):
            nc.tensor.matmul(wups[:], wuw[:], wux[:], start=True, stop=True)

        # first_matmul first (higher scheduler priority for its ACT
        # normalize ops); the preamble's ktf DMA issues are dep-free so the
        # scheduler still hoists them into the Act engine's idle start.
        vcur_pending = {0: first_matmul(0)}
        emit_preamble()

        for c in range(NCH):
            vcur = vcur_pending.pop(c)
            # ---------------- steps 1..STEPS-1 (normal orientation) ------
            for s in range(1, STEPS):
                vnext = {}
                for it in range(NT):
                    ps = [psum.tile([P, CH], f32, name=f"ps{s}_{c}_{it}_{d}",
                                    tag=f"ps{d}", bufs=2) for d in range(D)]
                    for j in range(NT):
                        for d in range(D):
                            nc.tensor.matmul(
                                ps[d][:],
                                ktb[j][:, it * P:(it + 1) * P],
                                vcur[d][j][:],
                                start=(j == 0), stop=(j == NT - 1),
                            )
                    bias_aps = (None if omega_zero else
                                [omc[it][:, d:d + 1] for d in range(D)])

                    def vouts(d, inv, _ps=ps, _it=it, _s=s):
                        vt = v_tile(d, _it, _s)
                        vnext.setdefault(d, {})[_it] = vt
                        if omega_zero:
                            nc.vector.tensor_tensor(
                                vt[:], _ps[d][:], inv[:], op=Op.mult)
                        else:
                            nc.vector.scalar_tensor_tensor(
                                vt[:], _ps[d][:], omc[_it][:, d:d + 1], inv[:],
                                op0=Op.add, op1=Op.mult,
                            )

                    normalize_unit([p[:] for p in ps], bias_aps, vouts, CH,
                                   f"s{s}_{c}_{it}")
                vcur = vnext

            # chunk c+1's first matmul emitted here: its matmuls fill the
            # last-step drain stalls, and its v0 slots are free by now.
            if c + 1 < NCH:
                vcur_pending[c + 1] = first_matmul(c + 1)

            # ---------------- last step, transposed: u[b, i] --------------
            # final chunk: the final b-tile (bt=2) is split into quarter
            # units; its first two (staged) units run EARLY -- right after
            # bt=3 -- so only the two slim quarter units remain at the very
            # end, and their normalize is all the un-overlappable drain.
            # (seq entries: bt, col, NW, slim, last_unit)
            if c == NCH - 1:
                seq = ([(bt, col, 512, False, False)
                        for bt in (3, 0, 1) for col in (0, 512)] +
                       [(2, 0, 256, False, False), (2, 256, 256, False, False),
                        (2, 512, 256, False, False),
                        (2, 768, 128, True, False), (2, 896, 128, True, True)])
            else:
                seq = [(bt, col, 512, False, False)
                       for bt in range(CH // P) for col in (0, 512)]
            for ic, (bt, col, NW, slim, last_unit) in enumerate(seq):
                    ps = [psum.tile([P, NW], f32, name=f"psL_{c}_{bt}_{ic}_{d}",
                                    tag=f"ps{d}", bufs=2) for d in range(D)]
                    for j in range(NT):
                        for d in range(D):
                            nc.tensor.matmul(
                                ps[d][:],
                                vcur[d][j][:, bt * P:(bt + 1) * P],
                                ktb[j][:, col:col + NW],
                                start=(j == 0),
                                stop=(omega_zero and j == NT - 1),
                            )
                    if not omega_zero:
                        for d in range(D):
                            nc.tensor.matmul(
                                ps[d][:],
                                ones1_box[0][:],
                                omrb[d][:, col:col + NW],
                                start=False, stop=True,
                            )

                    if slim:
                        u = ps
                        nsrc = [p[:] for p in ps]
                    else:
                        # stage u out of PSUM early so the banks free in
                        # ~1.7us instead of being held through the V writes.
                        # On DVE, not ACT: the last step's ACT chain (4x
                        # Square + Ln + Exp per unit) is what the end-of-
                        # kernel drain serializes on. The WHOLE normalize
                        # (Squares included) reads u, so the copies are the
                        # only PSUM readers -- otherwise the next-next
                        # unit's matmuls wait on the (late, DVE-gated) ACT
                        # Squares for bank release.
                        u = [pool.tile([P, NW], f32,
                                       name=f"u{d}_L{c}_{bt}_{ic}",
                                       tag=f"u{d}", bufs=2) for d in range(D)]
                        for d in range(D):
                            nc.vector.tensor_copy(u[d][:], ps[d][:])
                        nsrc = [t[:] for t in u]

                    if last_unit:
                        # split the very last output DMA per d-pair so the
                        # first half's transfer overlaps the second half's
                        # multiplies.
                        Vp = [pool.tile([P, NW * 2], f32,
                                        name=f"Vp{pr}_{c}_{bt}_{ic}",
                                        tag=f"bigh{pr}", bufs=1)
                              for pr in range(2)]

                        def voutL(d, inv, _Vp=Vp, _u=u, _NW=NW):
                            nc.vector.tensor_tensor(
                                _Vp[d // 2][:, (d % 2) * _NW:(d % 2 + 1) * _NW],
                                _u[d][:], inv[:], op=Op.mult)

                        normalize_unit(nsrc, None, voutL, NW,
                                       f"L{c}_{bt}_{ic}")
                        obase = (out[(c * CH + bt * P):(c * CH + (bt + 1) * P), :]
                                 .rearrange("r (d o) -> r d o", d=D))
                        for pr in range(2):
                            nc.sync.dma_start(
                                obase[:, 2 * pr:2 * pr + 2, col:col + NW],
                                Vp[pr][:].rearrange("p (d n) -> p d n", d=2),
                            )
                    else:
                        V = pool.tile([P, NW * D], f32, name=f"V{c}_{bt}_{ic}",
                                      tag="big", bufs=2)

                        def voutL(d, inv, _V=V, _u=u, _NW=NW):
                            nc.vector.tensor_tensor(
                                _V[:, d * _NW:(d + 1) * _NW], _u[d][:], inv[:],
                                op=Op.mult,
                            )

                        normalize_unit(nsrc, None, voutL, NW,
                                       f"L{c}_{bt}_{ic}")
                        nc.sync.dma_start(
                            out[(c * CH + bt * P):(c * CH + (bt + 1) * P), :]
                            .rearrange("r (d o) -> r d o", d=D)[:, :, col:col + NW],
                            V[:].rearrange("p (d n) -> p d n", d=D),
                        )

    nc.finalize()
    if os.environ.get("KERNEL_FUSE") == "1":
        _fuse_ldweights_json(nc)
    elif os.environ.get("KERNEL_NODEDUP") != "1":
        _dedup_ldweights_json(nc)
    return nc


_CACHED = {}


def kernel(x, W_in, omega, coupling):
    _install_hook_shim()
    _patch_ldw_opt()
    from concourse.bass_utils import run_bass_kernel_spmd

    x = np.ascontiguousarray(np.asarray(x, dtype=np.float32))
    W_in = np.asarray(W_in, dtype=np.float32)
    omega = np.ascontiguousarray(np.asarray(omega, dtype=np.float32))
    coupling = np.asarray(coupling, dtype=np.float32)

    # wh[ot*2+h, p, itl*D*P + d*P + o] = W_in[(4h+itl)*P+p, ot*P+o, d]
    wt = W_in.transpose(2, 0, 1).astype(np.float16)                # [D, IN, OUT]
    wh_host = np.ascontiguousarray(
        wt.reshape(D, NT, P, NT, P)          # [d, it, p, ot, o]
        .transpose(3, 1, 2, 0, 4)            # [ot, it, p, d, o]
        .reshape(NT, 2, NT // 2, P, D, P)    # [ot, h, itl, p, d, o]
        .transpose(0, 1, 3, 2, 4, 5)         # [ot, h, p, itl, d, o]
        .reshape(NT * 2, P, (NT // 2) * D * P))
    ct_host = np.ascontiguousarray(coupling.T.astype(np.float16))  # [OUT, OUT]
    omr_host = np.ascontiguousarray(omega.T)                       # [D, OUT]
    eye_host = np.eye(P, dtype=np.float16)

    omega_zero = not np.any(omega)
    key = ("nc", omega_zero)
    if key not in _CACHED:
        _CACHED[key] = _build(omega_zero)
    nc = _CACHED[key]

    in_maps = []
    for core in range(NCORES):
        xs = x[core * BS:(core + 1) * BS, :]
        # xh[c*2+hx, p, tl*CH + b] = xs[c*CH+b, (4hx+tl)*P+p]
        xh_host = np.ascontiguousarray(
            xs.T.astype(np.float16)
            .reshape(2, NT // 2, P, NCH, CH)     # [hx, tl, p, c, b]
            .transpose(3, 0, 2, 1, 4)            # [c, hx, p, tl, b]
            .reshape(NCH * 2, P, (NT // 2) * CH))
        im = {
            "xh": xh_host,
            "wh": wh_host,
            "ct": ct_host,
            "eye_in": eye_host,
        }
        if not omega_zero:
            im["om"] = omega
            im["omr"] = omr_host
        in_maps.append(im)

    trace = os.environ.get("KERNEL_TRACE") == "1"
    res = run_bass_kernel_spmd(nc, in_maps, core_ids=list(range(NCORES)), trace=trace)
    if trace and res.exec_time_ns:
        print(f"HW exec time: {res.exec_time_ns} ns")
        _CACHED["exec_time_ns"] = res.exec_time_ns
        _CACHED["results"] = res

    outs = [res.results[i]["out"].reshape(BS, D, OUT).transpose(0, 2, 1)
            for i in range(NCORES)]
    return np.ascontiguousarray(np.concatenate(outs, axis=0))



# revision 44
# speedup vs baseline: 1.0095x; 1.0037x over previous
"""AKOrN layer on 8 TRN2 NeuronCores, data-parallel over batch.

reference: v = l2norm_d(x @ W_in); K = tanh(coupling);
           8x: v = l2norm_d(v + K @ v + omega); return v [B, OUT, D]

Implementation notes:
- Data-parallel: batch 8192 -> 1024 rows per core; W_in/coupling/omega
  replicated. No collectives.
- K' = tanh(coupling) + I folds the "+ v" into the step matmul, so each step
  is pure matmul work plus a PSUM-side normalize.
- v lives on-chip as 4 per-d planes [OUT(part), batch(free)] in fp16 (8x
  finer mantissa than bf16 at identical PE speed; bf16 landed at rel err
  2.4e-2, fp16 at 3.3e-3). Batch is processed in 2 sequential 512-column
  chunks (SBUF fit for the double-buffered v generations).
- Step: 8 j-tiles x 4 d matmuls accumulate K'^T @ v_d into 4 PSUM banks
  (2 normalize units in flight = all 8 banks), then:
    q_d = Square(psum_d + omega_d)        (ACT, per-partition bias, fp16 out)
    s   = q0+q1+q2+q3                     (DVE fp16, 2x mode)
    inv = Exp(-0.5 * Ln(s))               (ACT, == rsqrt(s), one act table)
    v'_d = (psum_d + omega_d) * inv       (DVE scalar_tensor_tensor -> fp16)
- Last step runs transposed (stationary = v-slice, moving = K'^T rows) to
  produce [batch, OUT] so the d-interleave + output DMA is contiguous;
  omega enters there via a K=1 ones-row matmul (skipped when omega == 0).
  u is staged out of PSUM by ACT copies so banks free early.
- chunk1's first matmul is emitted between chunk0's steps and last step:
  its matmuls fill the last-step PSUM-drain stalls.
- x^T and W_in are uploaded pre-transposed/de-interleaved in fp16 (host-side
  layout marshalling only; all model arithmetic runs on device). Host
  layouts are arranged so every input DMA is fully contiguous (8KB/
  partition lines) -- 256B strided descriptors starved the PE during the
  first matmul (all dynamic DMAs share one SP HW-DGE queue, ~220 GB/s).
- Output is written d-major ([BS, D*OUT], host transposes): the last step's
  V writes are then contiguous DVE ops; d-interleaved strided writes
  measured ~2.3x slower and serialized the end-of-kernel drain.
- Last step runs b-tile order [3,0,1,2] per chunk-1 so the final tile's
  normalize overlaps other tiles' matmuls; u-staging copies run on DVE and
  the whole normalize reads the staged u (banks free right after the
  copies); the final tile is 3x NW=256 staged + 2x NW=128 slim units, and
  the very last unit's output DMA is split per d-pair so the transfer
  overlaps the remaining multiplies.
- Input staging: W streams on the SP HWDGE queue in it-major 0.5MB halves
  (bufs=8); x rides the Act HWDGE queue in halves; the 2MB coupling load is
  WAW-gated on W-half #9 landing so it cannot steal early bandwidth (the
  16 shared DMA engines round-robin packets fairly across ALL in-flight
  descriptors -- K outstanding tiles each get BW/K, so extra early
  descriptors make the first W tiles late, stall the PE at ot2..6, and
  re-throttle HAM). ktf bufs=8 keeps every coupling DMA issue ungated (a
  gated DMA-issue blocks later compute ops in that engine's stream).
- qPoolDynamic (unused SWDGE queue set) is dropped from nc.m.queues
  (50 -> 34 allocated queues); the end-of-NEFF teardown (~9us of
  per-semaphore waits on every engine) is otherwise a fixed cost.
- Tile pre-splits every Matmult into Ldweights+Matmult; a BIR-JSON post-pass
  drops Ldweights that reload the identical stationary (the d-loop reuses
  each K' tile 4x), and bacc's act-table pass is disabled in favor of
  walrus lower_act (bacc's greedy alternated two tables 289x per kernel).
Measured: ~1.034-1.037 ms HW exec (run-to-run spread; occasional runs hit
a chip-level P0 power downclock to 2.0 GHz and read ~1.2 ms), rel err
3.45e-3 (gate 2e-2); PE busy ~97%; fp16 streaming floor for the ~4600-MM
schedule is ~994 us + ~10 us prologue/HAM warm-up + ~17 us drain+teardown.
fp8/DoubleRow was evaluated and rejected: the dynamics AMPLIFY injected
quantization error ~1.2x/step (numpy+ml_dtypes sim: one e4m3 step in the
last position alone gives 4.1e-2 > gate; all-fp16 sim 2.8e-3 matches HW).
Keep per-d PSUM tiles: Tile's WAR tracking is whole-tile, so a merged
4-bank tile makes unit k+2's matmuls wait the last d-mul (+0.7us/unit).
"""
import contextlib
import ctypes
import os
import sys
import types

import numpy as np

B, IN, OUT, D = 8192, 1024, 1024, 4
STEPS = 8
NCORES = 8
BS = B // NCORES      # batch shard per core = 1024
CH = 512              # on-chip batch chunk (2 chunks, processed serially)
NCH = BS // CH
P = 128
NT = OUT // P         # 8 partition tiles

_SO_PATH = "/opt/axon/libaxon_pjrt.so"


# ---------------------------------------------------------------- plumbing
def _ntff_profile_via_ctypes(so_path):
    try:
        lib = ctypes.CDLL(so_path)
    except OSError:
        return None
    if not hasattr(lib, "axon_start_nrt_profile"):
        return None
    lib.axon_start_nrt_profile.argtypes = [ctypes.POINTER(ctypes.c_int64), ctypes.c_size_t]
    lib.axon_start_nrt_profile.restype = ctypes.c_int64
    lib.axon_stop_nrt_profile.argtypes = [ctypes.c_char_p]
    lib.axon_stop_nrt_profile.restype = ctypes.c_int64

    @contextlib.contextmanager
    def _hook(output_dir, device_ids):
        import jax

        jax.devices()
        if device_ids:
            ids = (ctypes.c_int64 * len(device_ids))(*device_ids)
            rc = lib.axon_start_nrt_profile(ids, len(device_ids))
        else:
            rc = lib.axon_start_nrt_profile(None, 0)
        if rc != 0:
            raise RuntimeError(f"axon_start_nrt_profile rc={rc}")
        try:
            yield
        finally:
            n = lib.axon_stop_nrt_profile(str(output_dir).encode())
            print(f"profile: {n} file(s) written to {output_dir}", file=sys.stderr)

    return _hook


def _install_hook_shim():
    if "antenv.axon_hooks" in sys.modules:
        return
    try:
        import antenv
    except ImportError:
        return
    mod = types.ModuleType("antenv.axon_hooks")
    _state = {"hook": _ntff_profile_via_ctypes(_SO_PATH)}
    mod.set_axon_ntff_profile_hook = lambda h: _state.__setitem__("hook", h)
    mod.get_axon_ntff_profile_hook = lambda: _state["hook"]
    sys.modules["antenv.axon_hooks"] = mod
    antenv.axon_hooks = mod


def _patch_ldw_opt():
    import concourse.bass_utils as bu

    if os.environ.get("KERNEL_FUSE") != "1":
        return
    if getattr(bu, "_ldw_patched", False):
        return
    orig = bu.run_command

    def patched(argv, **kwargs):
        argv = [
            a.replace("--enable-ldw-opt=false", "--enable-ldw-opt=true")
            if isinstance(a, str)
            else a
            for a in argv
        ]
        return orig(argv, **kwargs)

    bu.run_command = patched
    bu._ldw_patched = True


def _dedup_ldweights_json(nc):
    """Drop Ldweights that reload the exact weights already resident
    (identical operand AP as previous Ldweights, only Matmults between)."""
    import orjson

    orig = nc.to_json_bytes

    def patched():
        bir = orjson.loads(orig())
        n = 0
        for f in bir.get("functions", []):
            for blk in f.get("blocks", []):
                insts = blk.get("instructions")
                if not insts:
                    continue
                keep = []
                last_w = None
                for ins in insts:
                    op = ins.get("opcode")
                    if op == "Ldweights":
                        si = ins.get("sync_info") or {}
                        wts = si.get("on_wait") or []
                        key = orjson.dumps(ins.get("ins"))
                        if key == last_w and not wts:
                            n += 1
                            continue
                        last_w = key
                        keep.append(ins)
                    elif op == "Matmult":
                        keep.append(ins)
                    else:
                        last_w = None
                        keep.append(ins)
                blk["instructions"] = keep
        return orjson.dumps(bir)

    nc.to_json_bytes = patched
    return nc


def _fuse_ldweights_json(nc):
    import orjson

    orig = nc.to_json_bytes

    def patched():
        bir = orjson.loads(orig())
        for f in bir.get("functions", []):
            for blk in f.get("blocks", []):
                insts = blk.get("instructions")
                if not insts:
                    continue
                keep = []
                for ins in insts:
                    if ins.get("opcode") == "Ldweights":
                        si = ins.get("sync_info") or {}
                        w = si.get("on_wait") or []
                        assert not (si.get("on_update") or []), ins["name"]
                        if w:
                            keep.append({
                                "opcode": "NoOp",
                                "name": ins["name"],
                                "engine": ins.get("engine", "PE"),
                                "ins": [],
                                "outs": [],
                                "sync_info": {"on_wait": w, "on_update": []},
                            })
                        continue
                    keep.append(ins)
                blk["instructions"] = keep
        return orjson.dumps(bir)

    nc.to_json_bytes = patched
    return nc


# ---------------------------------------------------------------- builder
def _build(omega_zero):
    import concourse.bacc as bacc
    import concourse.mybir as mybir
    from concourse import tile

    A = mybir.ActivationFunctionType
    Op = mybir.AluOpType
    f32 = mybir.dt.float32
    bf16 = mybir.dt.float16  # fp16: 8x finer mantissa than bf16, same PE speed

    class BaccNoSplit(bacc.Bacc):
        def move_matmul_waits_to_ldweights(self):
            return

        def insert_act_table_loads(self):
            # walrus lower_act picks act-func sets globally (bacc's greedy
            # alternates natural_log/exp_and_others per normalize unit,
            # 289 table reloads)
            return

    nc = BaccNoSplit(None, target_bir_lowering=False)

    if os.environ.get("KERNEL_KEEPQ") != "1":
        # qPoolDynamic (SWDGE) is unused -- memset is an engine op, and all
        # DMAs go through the two HWDGE queues. Dropping it shrinks the
        # end-of-NEFF teardown, which waits per allocated physical queue.
        nc.m.queues = [q for q in nc.m.queues if q.name != "qPoolDynamic"]

    # xh[c*2+hx, p, tl*CH+b] = x[c*CH+b, (4hx+tl)*P+p]; wh[ot*4+qh, p,
    # itl*D*P+d*P+o] = W_in[(2qh+itl)*P+p, ot*P+o, d] -- both DMA as fully
    # contiguous blocks (4KB/2KB per-partition lines) instead of 256B
    # strided descriptors. W rides in 0.25MB quarters (bufs=16) and x in
    # 0.5MB halves: the 16 shared DMA engines round-robin across all
    # outstanding descriptors, so smaller descriptors complete sooner and
    # the PE's per-tile need times are met (1MB tiles starved the PE at
    # ot=3..6 and re-throttled HAM).
    xh = nc.declare_dram_parameter("xh", [NCH * 2, P, (NT // 2) * CH], mybir.dt.float16, isOutput=False)
    wh = nc.declare_dram_parameter("wh", [NT * 2, P, (NT // 2) * D * P], mybir.dt.float16, isOutput=False)
    ct = nc.declare_dram_parameter("ct", [OUT, OUT], mybir.dt.float16, isOutput=False)
    if not omega_zero:
        om = nc.declare_dram_parameter("om", [OUT, D], f32, isOutput=False)
        omr = nc.declare_dram_parameter("omr", [D, OUT], f32, isOutput=False)
    eye_in = nc.declare_dram_parameter("eye_in", [P, P], mybir.dt.float16, isOutput=False)
    # d-major output layout [BS, D*OUT]: every DVE write in the last step
    # is contiguous (strided d-interleave writes measured ~2.3x slower) and
    # the DMA still moves 2KB runs. Host transposes to [BS, OUT, D].
    out = nc.declare_dram_parameter("out", [BS, D * OUT], f32, isOutput=True)

    with tile.TileContext(nc) as tc, contextlib.ExitStack() as ctx:
        const = ctx.enter_context(tc.tile_pool(name="const", bufs=1))
        pool = ctx.enter_context(tc.tile_pool(name="pool", bufs=1))
        psum = ctx.enter_context(tc.tile_pool(name="psum", bufs=1, space="PSUM"))

        omc = []
        omrb = []
        ktb = []
        ones1_box = []
        wb_gate = []  # chunk-0 W-half tiles; emit_preamble gates ktf on one

        def emit_preamble():
            # constants / K' = tanh(ct)^T rows + I. Emitted AFTER chunk 0's
            # first matmul so the PE starts as soon as xt/wb land. All
            # preamble DMAs ride the Activation HWDGE queue so they never
            # contend with the W stream on the SP queue.
            eyeb = const.tile([P, P], bf16, name="eyeb", tag="eyeb")
            nc.scalar.dma_start(eyeb[:], eye_in[:])

            if not omega_zero:
                for t in range(NT):  # omega columns per o-tile: [128, D] fp32
                    o = const.tile([P, D], f32, name=f"omc{t}", tag=f"omc{t}")
                    nc.sync.dma_start(o[:], om[t * P:(t + 1) * P, :])
                    omc.append(o)
                for d in range(D):  # omega rows per d: [1, OUT] fp16
                    of = const.tile([1, OUT], f32, name=f"omrf{d}", tag=f"omrf{d}")
                    nc.sync.dma_start(of[:], omr[d:d + 1, :])
                    ob = const.tile([1, OUT], bf16, name=f"omrb{d}", tag=f"omrb{d}")
                    nc.vector.tensor_copy(ob[:], of[:])
                    omrb.append(ob)
                ones1 = const.tile([1, P], bf16, name="ones1", tag="ones1")
                nc.gpsimd.memset(ones1[:], 1.0)
                ones1_box.append(ones1)

            for j in range(NT):  # K'^T tiles: [128(j), OUT(i)] fp16
                # ktf rides the SP queue, emitted AFTER all 16 W-half DMAs:
                # its issues queue up behind the MM-gated W issues, so the
                # 2MB of coupling data only moves at ~40-55us -- leaving the
                # early window's full bandwidth to W (W tiles arriving
                # just-late starved the PE at ot2..6 and re-throttled HAM).
                # K' is still ready well before step 1 needs it at ~66us.
                # bufs=8: every ktf DMA issue is ungated by Tanh progress (a
                # gated issue would block later compute in an engine stream).
                kf = pool.tile([P, OUT], bf16, name=f"ktf{j}", tag="ktf", bufs=8)
                if wb_gate:
                    # WAW-gate the ktf DMA on W-half #9 having LANDED (a
                    # 1-element DVE write into kf forces the ordering): the
                    # scheduler otherwise hoists the dep-free ktf issues
                    # into the early window where their 2MB starves W.
                    nc.vector.tensor_copy(kf[:, 0:1], wb_gate[9][:, 0:1])
                nc.sync.dma_start(kf[:], ct[j * P:(j + 1) * P, :])
                kb = const.tile([P, OUT], bf16, name=f"ktb{j}", tag=f"ktb{j}")
                nc.scalar.activation(kb[:], kf[:], A.Tanh)
                nc.vector.tensor_tensor(
                    kb[:, j * P:(j + 1) * P], kb[:, j * P:(j + 1) * P], eyeb[:],
                    op=Op.add,
                )
                ktb.append(kb)

        # v planes: tag per (d, j), 2 bufs (generation ping-pong)
        def v_tile(d, j, s):
            return const.tile([P, CH], bf16, name=f"v_s{s}_d{d}_j{j}",
                             tag=f"v{d}_{j}", bufs=2)

        def normalize_unit(ps, bias_aps, vout, n, label):
            """ps: 4 psum APs [P,n] (separate per-d tiles: Tile's WAR
            tracking is whole-tile, so per-d tiles give the staggered
            bank release the next-next unit's d-ordered matmuls need --
            a merged 4-bank tile measured +0.7us PE stall per unit).
            bias_aps: 4 per-part scalars or None; vout(d, inv) -> emits
            the final scaled write for plane d."""
            q = [pool.tile([P, n], bf16, name=f"q{d}_{label}", tag=f"q{d}", bufs=2)
                 for d in range(D)]
            for d in range(D):
                if bias_aps is None:
                    nc.scalar.activation(q[d][:], ps[d], A.Square)
                else:
                    nc.scalar.activation(q[d][:], ps[d], A.Square, bias=bias_aps[d])
            s01 = pool.tile([P, n], bf16, name=f"s01_{label}", tag="s01", bufs=2)
            s23 = pool.tile([P, n], bf16, name=f"s23_{label}", tag="s23", bufs=2)
            ssum = pool.tile([P, n], bf16, name=f"ss_{label}", tag="ss", bufs=2)
            nc.vector.tensor_tensor(s01[:], q[0][:], q[1][:], op=Op.add)
            nc.vector.tensor_tensor(s23[:], q[2][:], q[3][:], op=Op.add)
            nc.vector.tensor_tensor(ssum[:], s01[:], s23[:], op=Op.add)
            lns = pool.tile([P, n], f32, name=f"ln_{label}", tag="lns", bufs=2)
            nc.scalar.activation(lns[:], ssum[:], A.Ln)
            inv = pool.tile([P, n], f32, name=f"inv_{label}", tag="inv", bufs=2)
            nc.scalar.activation(inv[:], lns[:], A.Exp, scale=-0.5)
            for d in range(D):
                vout(d, inv)

        def first_matmul(c):
            # v0 = l2norm(x @ W_in). x rides the Act queue in two halves
            # (needed first, lands in parallel with the W stream on the SP
            # queue; the first matmuls only need half 0).
            # bufs=2: chunk 1's halves WAR-wait on chunk 0's being fully
            # consumed (~63us) -- with bufs=4 the scheduler hoisted chunk
            # 1's 1MB into the bandwidth-critical first 15us.
            xth = []
            for hx in range(2):
                t = pool.tile([P, (NT // 2) * CH], bf16,
                              name=f"xt{c}_{hx}", tag="xt", bufs=2)
                nc.scalar.dma_start(t[:], xh[c * 2 + hx])
                xth.append(t)

            HIT = NT // 2  # it-tiles per W half
            vcur = {}
            for ot in range(NT):
                ps = [psum.tile([P, CH], f32, name=f"ps0_{c}_{ot}_{d}",
                                tag=f"ps{d}", bufs=2) for d in range(D)]
                for h in range(2):
                    wbh = pool.tile([P, HIT * D * P], bf16,
                                    name=f"wb{c}_{ot}_{h}", tag="wb", bufs=8)
                    nc.sync.dma_start(wbh[:], wh[ot * 2 + h])
                    if c == 0:
                        wb_gate.append(wbh)
                    for itl in range(HIT):
                        it = h * HIT + itl
                        for d in range(D):
                            nc.tensor.matmul(
                                ps[d][:],
                                wbh[:, itl * D * P + d * P:itl * D * P + (d + 1) * P],
                                xth[it // 4][:, (it % 4) * CH:(it % 4 + 1) * CH],
                                start=(it == 0), stop=(it == NT - 1),
                            )

                def vout0(d, inv, _ps=ps, _ot=ot, _c=c):
                    vt = v_tile(d, _ot, 0)
                    vcur.setdefault(d, {})[_ot] = vt
                    nc.vector.tensor_tensor(vt[:], _ps[d][:], inv[:], op=Op.mult)

                normalize_unit([p[:] for p in ps], None, vout0, CH, f"f{c}_{ot}")
            return vcur

        # PE warm-up: dummy matmuls on memset data fill the initial
        # input-DMA wait and lift the HAM clock gate to 2.4 GHz before the
        # first real matmul issues. Sized to bridge until the first weight
        # tile lands (~12.5us): a >3.4us PE-idle gap between warm-up and
        # the first real matmul re-throttles the clock to 1.2 GHz and the
        # whole first o-tile runs at half speed (seen as a HAM K=4/8 event
        # at ~15.6us in the trace). First ~8 run cold (~427ns), rest warm
        # (~214ns): 40 MMs end ~10.6us.
        wuw = pool.tile([P, P], bf16, name="wuw", tag="wuw")
        wux = pool.tile([P, 512], bf16, name="wux", tag="wux")
        nc.gpsimd.memset(wuw[:], 0.0)
        nc.gpsimd.memset(wux[:], 0.0)
        wups = psum.tile([P, 512], f32, name="wups", tag="ps0", bufs=2)
        for _ in range(28):
            nc.tensor.matmul(wups[:], wuw[:], wux[:], start=True, stop=True)

        # first_matmul first (higher scheduler priority for its ACT
        # normalize ops); the preamble's ktf DMA issues are dep-free so the
        # scheduler still hoists them into the Act engine's idle start.
        vcur_pending = {0: first_matmul(0)}
        emit_preamble()

        for c in range(NCH):
            vcur = vcur_pending.pop(c)
            # ---------------- steps 1..STEPS-1 (normal orientation) ------
            for s in range(1, STEPS):
                vnext = {}
                for it in range(NT):
                    ps = [psum.tile([P, CH], f32, name=f"ps{s}_{c}_{it}_{d}",
                                    tag=f"ps{d}", bufs=2) for d in range(D)]
                    for j in range(NT):
                        for d in range(D):
                            nc.tensor.matmul(
                                ps[d][:],
                                ktb[j][:, it * P:(it + 1) * P],
                                vcur[d][j][:],
                                start=(j == 0), stop=(j == NT - 1),
                            )
                    bias_aps = (None if omega_zero else
                                [omc[it][:, d:d + 1] for d in range(D)])

                    def vouts(d, inv, _ps=ps, _it=it, _s=s):
                        vt = v_tile(d, _it, _s)
                        vnext.setdefault(d, {})[_it] = vt
                        if omega_zero:
                            nc.vector.tensor_tensor(
                                vt[:], _ps[d][:], inv[:], op=Op.mult)
                        else:
                            nc.vector.scalar_tensor_tensor(
                                vt[:], _ps[d][:], omc[_it][:, d:d + 1], inv[:],
                                op0=Op.add, op1=Op.mult,
                            )

                    normalize_unit([p[:] for p in ps], bias_aps, vouts, CH,
                                   f"s{s}_{c}_{it}")
                vcur = vnext

            # chunk c+1's first matmul emitted here: its matmuls fill the
            # last-step drain stalls, and its v0 slots are free by now.
            if c + 1 < NCH:
                vcur_pending[c + 1] = first_matmul(c + 1)

            # ---------------- last step, transposed: u[b, i] --------------
            # final chunk: the final b-tile (bt=2) is split into quarter
            # units; its first two (staged) units run EARLY -- right after
            # bt=3 -- so only the two slim quarter units remain at the very
            # end, and their normalize is all the un-overlappable drain.
            # (seq entries: bt, col, NW, slim, last_unit)
            if c == NCH - 1:
                seq = ([(bt, col, 512, False, False)
                        for bt in (3, 0, 1) for col in (0, 512)] +
                       [(2, 0, 256, False, False), (2, 256, 256, False, False),
                        (2, 512, 256, True, False), (2, 768, 256, True, True)])
            else:
                seq = [(bt, col, 512, False, False)
                       for bt in range(CH // P) for col in (0, 512)]
            for ic, (bt, col, NW, slim, last_unit) in enumerate(seq):
                    ps = [psum.tile([P, NW], f32, name=f"psL_{c}_{bt}_{ic}_{d}",
                                    tag=f"ps{d}", bufs=2) for d in range(D)]
                    for j in range(NT):
                        for d in range(D):
                            nc.tensor.matmul(
                                ps[d][:],
                                vcur[d][j][:, bt * P:(bt + 1) * P],
                                ktb[j][:, col:col + NW],
                                start=(j == 0),
                                stop=(omega_zero and j == NT - 1),
                            )
                    if not omega_zero:
                        for d in range(D):
                            nc.tensor.matmul(
                                ps[d][:],
                                ones1_box[0][:],
                                omrb[d][:, col:col + NW],
                                start=False, stop=True,
                            )

                    if slim:
                        u = ps
                        nsrc = [p[:] for p in ps]
                    else:
                        # stage u out of PSUM early so the banks free in
                        # ~1.7us instead of being held through the V writes.
                        # On DVE, not ACT: the last step's ACT chain (4x
                        # Square + Ln + Exp per unit) is what the end-of-
                        # kernel drain serializes on. The WHOLE normalize
                        # (Squares included) reads u, so the copies are the
                        # only PSUM readers -- otherwise the next-next
                        # unit's matmuls wait on the (late, DVE-gated) ACT
                        # Squares for bank release.
                        u = [pool.tile([P, NW], f32,
                                       name=f"u{d}_L{c}_{bt}_{ic}",
                                       tag=f"u{d}", bufs=2) for d in range(D)]
                        for d in range(D):
                            nc.vector.tensor_copy(u[d][:], ps[d][:])
                        nsrc = [t[:] for t in u]

                    if last_unit:
                        # split the very last output DMA per d-pair so the
                        # first half's transfer overlaps the second half's
                        # multiplies.
                        Vp = [pool.tile([P, NW * 2], f32,
                                        name=f"Vp{pr}_{c}_{bt}_{ic}",
                                        tag=f"bigh{pr}", bufs=1)
                              for pr in range(2)]

                        def voutL(d, inv, _Vp=Vp, _u=u, _NW=NW):
                            nc.vector.tensor_tensor(
                                _Vp[d // 2][:, (d % 2) * _NW:(d % 2 + 1) * _NW],
                                _u[d][:], inv[:], op=Op.mult)

                        normalize_unit(nsrc, None, voutL, NW,
                                       f"L{c}_{bt}_{ic}")
                        obase = (out[(c * CH + bt * P):(c * CH + (bt + 1) * P), :]
                                 .rearrange("r (d o) -> r d o", d=D))
                        for pr in range(2):
                            nc.sync.dma_start(
                                obase[:, 2 * pr:2 * pr + 2, col:col + NW],
                                Vp[pr][:].rearrange("p (d n) -> p d n", d=2),
                            )
                    else:
                        V = pool.tile([P, NW * D], f32, name=f"V{c}_{bt}_{ic}",
                                      tag="big", bufs=2)

                        def voutL(d, inv, _V=V, _u=u, _NW=NW):
                            nc.vector.tensor_tensor(
                                _V[:, d * _NW:(d + 1) * _NW], _u[d][:], inv[:],
                                op=Op.mult,
                            )

                        normalize_unit(nsrc, None, voutL, NW,
                                       f"L{c}_{bt}_{ic}")
                        nc.sync.dma_start(
                            out[(c * CH + bt * P):(c * CH + (bt + 1) * P), :]
                            .rearrange("r (d o) -> r d o", d=D)[:, :, col:col + NW],
                            V[:].rearrange("p (d n) -> p d n", d=D),
                        )

    nc.finalize()
    if os.environ.get("KERNEL_FUSE") == "1":
        _fuse_ldweights_json(nc)
    elif os.environ.get("KERNEL_NODEDUP") != "1":
        _dedup_ldweights_json(nc)
    return nc


_CACHED = {}


def kernel(x, W_in, omega, coupling):
    _install_hook_shim()
    _patch_ldw_opt()
    from concourse.bass_utils import run_bass_kernel_spmd

    x = np.ascontiguousarray(np.asarray(x, dtype=np.float32))
    W_in = np.asarray(W_in, dtype=np.float32)
    omega = np.ascontiguousarray(np.asarray(omega, dtype=np.float32))
    coupling = np.asarray(coupling, dtype=np.float32)

    # wh[ot*2+h, p, itl*D*P + d*P + o] = W_in[(4h+itl)*P+p, ot*P+o, d]
    wt = W_in.transpose(2, 0, 1).astype(np.float16)                # [D, IN, OUT]
    wh_host = np.ascontiguousarray(
        wt.reshape(D, NT, P, NT, P)          # [d, it, p, ot, o]
        .transpose(3, 1, 2, 0, 4)            # [ot, it, p, d, o]
        .reshape(NT, 2, NT // 2, P, D, P)    # [ot, h, itl, p, d, o]
        .transpose(0, 1, 3, 2, 4, 5)         # [ot, h, p, itl, d, o]
        .reshape(NT * 2, P, (NT // 2) * D * P))
    ct_host = np.ascontiguousarray(coupling.T.astype(np.float16))  # [OUT, OUT]
    omr_host = np.ascontiguousarray(omega.T)                       # [D, OUT]
    eye_host = np.eye(P, dtype=np.float16)

    omega_zero = not np.any(omega)
    key = ("nc", omega_zero)
    if key not in _CACHED:
        _CACHED[key] = _build(omega_zero)
    nc = _CACHED[key]

    in_maps = []
    for core in range(NCORES):
        xs = x[core * BS:(core + 1) * BS, :]
        # xh[c*2+hx, p, tl*CH + b] = xs[c*CH+b, (4hx+tl)*P+p]
        xh_host = np.ascontiguousarray(
            xs.T.astype(np.float16)
            .reshape(2, NT // 2, P, NCH, CH)     # [hx, tl, p, c, b]
            .transpose(3, 0, 2, 1, 4)            # [c, hx, p, tl, b]
            .reshape(NCH * 2, P, (NT // 2) * CH))
        im = {
            "xh": xh_host,
            "wh": wh_host,
            "ct": ct_host,
            "eye_in": eye_host,
        }
        if not omega_zero:
            im["om"] = omega
            im["omr"] = omr_host
        in_maps.append(im)

    trace = os.environ.get("KERNEL_TRACE") == "1"
    res = run_bass_kernel_spmd(nc, in_maps, core_ids=list(range(NCORES)), trace=trace)
    if trace and res.exec_time_ns:
        print(f"HW exec time: {res.exec_time_ns} ns")
        _CACHED["exec_time_ns"] = res.exec_time_ns
        _CACHED["results"] = res

    outs = [res.results[i]["out"].reshape(BS, D, OUT).transpose(0, 2, 1)
            for i in range(NCORES)]
    return np.ascontiguousarray(np.concatenate(outs, axis=0))

